# revision 8
# baseline (speedup 1.0000x reference)
"""GroupHadamardLayer (segment_reduce) Trainium2 kernel.

The reference computes, for arbitrary group_idx:
    gathered = x[:, group_idx]                # [B, 256, 8]
    h = einsum('bng,ng->bn', gathered, gc_w)  # [B, 256]
    h = h * diag_w
    out = h @ fc_w                            # [B, 1]

This is linear in x, so it collapses to out = x @ w with
    w[group_idx[n, g]] += gc_w[n, g] * diag_w[n] * fc_w[n, 0]
(scatter-add — exact for duplicate indices too).

Device kernel: pure memory-bound matvec. x [16384, 2048] f32 (128 MiB) is
sharded by batch across 8 cores (2048 rows / 16 MiB each). Each core
streams its shard as 16 contiguous 1 MiB row-group chunks [128, 2048].
Per row-group the dot products against the partition-replicated w are
reduced on-chip; deep tile pools keep the HBM stream free of compute
backpressure (the chunk-dispatch instruction on SyncE carries the
buffer-reuse wait, so shallow pools stall the DMA ring itself).
exec_time ~= last-flush-dispatch + 4.3 us (fixed epilogue), so the
kernel minimizes trailing compute after the final HBM byte.
"""

import os
import sys
from contextlib import ExitStack

sys.path.insert(0, "/opt/trn_rl_repo")

import numpy as np

from concourse import bacc, bass, tile
from concourse.bass_utils import run_bass_kernel_spmd

mybir = bass.mybir
F32 = mybir.dt.float32

B, F = 16384, 2048
N_CORES = 8
ROWS = B // N_CORES  # 2048 rows per core
P = 128
N_TILES = ROWS // P  # 16 row-groups of 1 MiB each

# Compute-stage variant, switchable for A/B experiments:
#   tt_act: TT multiply on VectorE + ACTIVATE accumulate on ScalarE (safe)
#   stt:    fused scalar_tensor_tensor w/ accum_out on VectorE
#   ttr:    fused tensor_tensor_reduce on VectorE
VARIANT = os.environ.get("KERNEL_VARIANT", "tt_act")

_NC = None
LAST_RESULT = None  # BassKernelResults of the most recent run (for test.py)


def _build_nc():
    # Bacc (not plain Bass): its finalize() runs generate_event_semaphores,
    # which splits multi-sem waits — TRN2 ISA allows 1 sync wait per inst.
    nc = bacc.Bacc("TRN2", target_bir_lowering=False, debug=False)
    x = nc.dram_tensor("x", [ROWS, F], F32, kind="ExternalInput")
    w = nc.dram_tensor("wrep", [P, F], F32, kind="ExternalInput")
    out = nc.dram_tensor("out", [P, N_TILES], F32, kind="ExternalOutput")

    with tile.TileContext(nc) as tc:
        with (
            tc.tile_pool(name="xp", bufs=10) as xp,
            tc.tile_pool(name="pp", bufs=8) as pp,
            tc.tile_pool(name="wp", bufs=1) as wp,
            tc.tile_pool(name="op", bufs=1) as op,
        ):
            # w arrives host-replicated to all 128 partitions (1 MiB). The
            # alternatives all lose: stride-0 DMA APs and GpSimd
            # partition_broadcast fail on this stack, and a TensorE K=1
            # broadcast (8 KB load + 8 fp32 matmuls + PSUM copy) finishes
            # ~4 us LATER than just streaming the 1 MiB (fp32 matmul is
            # quarter-rate and the cold 8 KB DMA alone takes ~5 us).
            w_t = wp.tile([P, F], F32)
            nc.sync.dma_start(w_t[:], w.ap())
            out_t = op.tile([P, N_TILES], F32)
            dummy = wp.tile([P, 1], F32)

            def reduce_rowgroup(x_ap, acc_ap):
                """acc_ap[p, 0] = sum_f x_ap[p, f] * w_t[p, f]."""
                if VARIANT == "stt":
                    nc.vector.scalar_tensor_tensor(
                        out=dummy.broadcast_to((P, F)),
                        in0=x_ap,
                        scalar=1.0,
                        in1=w_t[:],
                        op0=mybir.AluOpType.mult,
                        op1=mybir.AluOpType.mult,
                        accum_out=acc_ap,
                    )
                elif VARIANT == "ttr":
                    nc.vector.tensor_tensor_reduce(
                        out=dummy.broadcast_to((P, F)),
                        in0=x_ap,
                        in1=w_t[:],
                        scale=1.0,
                        scalar=0.0,
                        op0=mybir.AluOpType.mult,
                        op1=mybir.AluOpType.add,
                        accum_out=acc_ap,
                    )
                else:  # tt_act
                    prod = pp.tile([P, F], F32, tag="prod")
                    nc.vector.tensor_tensor(
                        out=prod[:],
                        in0=x_ap,
                        in1=w_t[:],
                        op=mybir.AluOpType.mult,
                    )
                    # ScalarE: dot product = sum_free(prod). out is a
                    # stride-0 dummy — only accum_out matters.
                    nc.scalar.activation(
                        out=dummy.broadcast_to((P, F)),
                        in_=prod[:],
                        func=mybir.ActivationFunctionType.Copy,
                        accum_out=acc_ap,
                    )

            for t in range(N_TILES):
                x_t = xp.tile([P, F], F32, tag="x")
                # rows [t*128, (t+1)*128): contiguous 1 MiB DRAM read
                nc.sync.dma_start(x_t[:], x.ap()[t * P : (t + 1) * P, :])
                reduce_rowgroup(x_t[:], out_t[:, t : t + 1])
            # Single flush at the end. An early half-flush is a trap: its
            # dispatch sits in the Sync queue carrying a wait on the ScalarE
            # accumulator reads, blocking every later chunk dispatch and
            # stalling the HBM stream for ~4 us.
            nc.sync.dma_start(out.ap(), out_t[:])
    nc.finalize()
    return nc


def kernel(x, group_idx, gc_w, diag_w, fc_w):
    global _NC, LAST_RESULT
    x = np.ascontiguousarray(np.asarray(x, dtype=np.float32))
    gi = np.asarray(group_idx).astype(np.int64)
    gc_w = np.asarray(gc_w, dtype=np.float32)
    diag_w = np.asarray(diag_w, dtype=np.float32).reshape(-1)
    fc_w = np.asarray(fc_w, dtype=np.float32).reshape(-1, 1)

    # Fold everything linear into one combined weight vector (exact).
    coef = gc_w * diag_w[:, None] * fc_w  # [256, 8]
    w = np.zeros(F, dtype=np.float32)
    np.add.at(w, gi.ravel(), coef.ravel().astype(np.float32))
    wrep = np.ascontiguousarray(np.broadcast_to(w, (P, F))).astype(np.float32)

    if _NC is None:
        _NC = _build_nc()

    in_maps = [
        {"x": np.ascontiguousarray(x[i * ROWS : (i + 1) * ROWS]), "wrep": wrep}
        for i in range(N_CORES)
    ]
    trace = bool(int(os.environ.get("TRN_KERNEL_TRACE", "0")))
    LAST_RESULT = run_bass_kernel_spmd(
        _NC, in_maps, list(range(N_CORES)), trace=trace
    )
    # out[p, t] is the dot product for shard row t*128 + p
    shard_outs = [
        LAST_RESULT.results[i]["out"].T.reshape(ROWS) for i in range(N_CORES)
    ]
    return np.concatenate(shard_outs).reshape(B, 1).astype(np.float32)


# revision 9
# speedup vs baseline: 1.1557x; 1.1557x over previous
"""GroupHadamardLayer (segment_reduce) Trainium2 kernel.

The reference computes, for arbitrary group_idx:
    gathered = x[:, group_idx]                # [B, 256, 8]
    h = einsum('bng,ng->bn', gathered, gc_w)  # [B, 256]
    h = h * diag_w
    out = h @ fc_w                            # [B, 1]

This is linear in x, so it collapses to out = x @ w with
    w[group_idx[n, g]] += gc_w[n, g] * diag_w[n] * fc_w[n, 0]
(scatter-add — exact for duplicate indices too).

Device kernel: pure memory-bound matvec. x [16384, 2048] f32 (128 MiB) is
sharded by batch across 8 cores (2048 rows / 16 MiB each). Each core
streams its shard as 16 contiguous 1 MiB row-group chunks [128, 2048]
into 16 DISTINCT SBUF buffers — zero buffer reuse, so no chunk dispatch
ever waits on compute (dispatch-on-SyncE carrying a compute wait starves
the DMA ring into a latency-bound lockstep). Per row-group: VectorE
multiply + ScalarE activation-accumulate; the LAST row-group instead uses
a single fused scalar_tensor_tensor on VectorE so only ~2.4 us of
compute trails the final HBM byte. w reaches SBUF as a 4 KB bf16 [1, F]
vector broadcast to 128 partitions by a TensorE K=1 matmul against ones
(PSUM), copied to SBUF by the otherwise-idle ScalarE — 1 MiB less HBM
traffic per core than streaming a host-replicated wrep.
exec_time ~= final-flush-dispatch + 4.3 us (fixed epilogue).
"""

import os
import sys
from contextlib import ExitStack

sys.path.insert(0, "/opt/trn_rl_repo")

import ml_dtypes
import numpy as np

from concourse import bacc, bass, tile
from concourse.bass_utils import run_bass_kernel_spmd

mybir = bass.mybir
F32 = mybir.dt.float32
BF16 = mybir.dt.bfloat16

B, F = 16384, 2048
N_CORES = 8
ROWS = B // N_CORES  # 2048 rows per core
P = 128
N_TILES = ROWS // P  # 16 row-groups of 1 MiB each
MM_N = 512  # TensorE moving-side max free dim

# A/B flags (defaults = best known config)
TAIL_STT = os.environ.get("KERNEL_TAIL_STT", "1") == "1"
WBCAST = os.environ.get("KERNEL_WBCAST", "0") == "1"

_NC = None
_NC_KEY = None
LAST_RESULT = None  # BassKernelResults of the most recent run (for test.py)


def _build_nc():
    # Bacc (not plain Bass): its finalize() runs generate_event_semaphores,
    # which splits multi-sem waits — TRN2 ISA allows 1 sync wait per inst.
    nc = bacc.Bacc("TRN2", target_bir_lowering=False, debug=False)
    x = nc.dram_tensor("x", [ROWS, F], F32, kind="ExternalInput")
    if WBCAST:
        wsm = nc.dram_tensor("wsm", [1, F], BF16, kind="ExternalInput")
    else:
        w = nc.dram_tensor("wrep", [P, F], F32, kind="ExternalInput")
    out = nc.dram_tensor("out", [P, N_TILES], F32, kind="ExternalOutput")

    with tile.TileContext(nc) as tc:
        with (
            tc.tile_pool(name="xp", bufs=N_TILES) as xp,
            tc.tile_pool(name="pp", bufs=5) as pp,
            tc.tile_pool(name="wp", bufs=1) as wp,
            tc.tile_pool(name="op", bufs=1) as op,
            ExitStack() as stack,
        ):
            w_t = wp.tile([P, F], F32)
            if WBCAST:
                # w arrives as 4 KB bf16 [1, F]; broadcast to 128 partitions
                # with a TensorE K=1 matmul (ones[1,P] stationary), 4 x 512
                # cols into 4 PSUM banks, each copied to SBUF by ScalarE
                # (idle this early). Ready ~11.5 us, before the first TT
                # needs it. Saves the 1 MiB host-replicated wrep stream.
                ps = stack.enter_context(
                    tc.tile_pool(name="ps", bufs=1, space="PSUM")
                )
                w_sb = wp.tile([1, F], BF16)
                nc.sync.dma_start(w_sb[:], wsm.ap())
                ones = wp.tile([1, P], BF16)
                nc.gpsimd.memset(ones[:], 1.0)
                psum_t = ps.tile([P, F // MM_N, MM_N], F32, space="PSUM")
                for k in range(F // MM_N):
                    nc.tensor.matmul(
                        psum_t[:, k, :],
                        ones[:],
                        w_sb[:, k * MM_N : (k + 1) * MM_N],
                        start=True,
                        stop=True,
                    )
                    nc.scalar.activation(
                        out=w_t[:, k * MM_N : (k + 1) * MM_N],
                        in_=psum_t[:, k, :],
                        func=mybir.ActivationFunctionType.Copy,
                    )
            else:
                # w host-replicated to all 128 partitions (1 MiB stream).
                nc.sync.dma_start(w_t[:], w.ap())
            out_t = op.tile([P, N_TILES], F32)
            dummy = wp.tile([P, 1], F32)

            for t in range(N_TILES):
                x_t = xp.tile([P, F], F32, tag="x")
                # rows [t*128, (t+1)*128): contiguous 1 MiB DRAM read
                nc.sync.dma_start(x_t[:], x.ap()[t * P : (t + 1) * P, :])
                if TAIL_STT and t == N_TILES - 1:
                    # Fused multiply+accumulate on VectorE: no ScalarE
                    # ACTIVATE+READ chain trailing the last chunk. (~2.37 us
                    # on DVE vs 2.29 — too slow to use for EVERY row-group,
                    # but ideal for the last one.)
                    nc.vector.scalar_tensor_tensor(
                        out=dummy.broadcast_to((P, F)),
                        in0=x_t[:],
                        scalar=1.0,
                        in1=w_t[:],
                        op0=mybir.AluOpType.mult,
                        op1=mybir.AluOpType.mult,
                        accum_out=out_t[:, t : t + 1],
                    )
                else:
                    prod = pp.tile([P, F], F32, tag="prod")
                    nc.vector.tensor_tensor(
                        out=prod[:],
                        in0=x_t[:],
                        in1=w_t[:],
                        op=mybir.AluOpType.mult,
                    )
                    # ScalarE: dot product = sum_free(prod). out is a
                    # stride-0 dummy — only accum_out matters.
                    nc.scalar.activation(
                        out=dummy.broadcast_to((P, F)),
                        in_=prod[:],
                        func=mybir.ActivationFunctionType.Copy,
                        accum_out=out_t[:, t : t + 1],
                    )
            # Single flush at the end. An early half-flush is a trap: its
            # dispatch sits in the Sync queue carrying a wait on the ScalarE
            # accumulator reads, blocking every later chunk dispatch.
            nc.sync.dma_start(out.ap(), out_t[:])
    nc.finalize()
    return nc


def kernel(x, group_idx, gc_w, diag_w, fc_w):
    global _NC, _NC_KEY, LAST_RESULT
    x = np.ascontiguousarray(np.asarray(x, dtype=np.float32))
    gi = np.asarray(group_idx).astype(np.int64)
    gc_w = np.asarray(gc_w, dtype=np.float32)
    diag_w = np.asarray(diag_w, dtype=np.float32).reshape(-1)
    fc_w = np.asarray(fc_w, dtype=np.float32).reshape(-1, 1)

    # Fold everything linear into one combined weight vector (exact).
    coef = gc_w * diag_w[:, None] * fc_w  # [256, 8]
    w = np.zeros(F, dtype=np.float32)
    np.add.at(w, gi.ravel(), coef.ravel().astype(np.float32))

    key = (TAIL_STT, WBCAST)
    if _NC is None or _NC_KEY != key:
        _NC = _build_nc()
        _NC_KEY = key

    if WBCAST:
        win = {"wsm": np.ascontiguousarray(w.reshape(1, F).astype(ml_dtypes.bfloat16))}
    else:
        win = {
            "wrep": np.ascontiguousarray(
                np.broadcast_to(w, (P, F)).astype(np.float32)
            )
        }
    in_maps = [
        {"x": np.ascontiguousarray(x[i * ROWS : (i + 1) * ROWS]), **win}
        for i in range(N_CORES)
    ]
    trace = bool(int(os.environ.get("TRN_KERNEL_TRACE", "0")))
    LAST_RESULT = run_bass_kernel_spmd(
        _NC, in_maps, list(range(N_CORES)), trace=trace
    )
    # out[p, t] is the dot product for shard row t*128 + p
    shard_outs = [
        LAST_RESULT.results[i]["out"].T.reshape(ROWS) for i in range(N_CORES)
    ]
    return np.concatenate(shard_outs).reshape(B, 1).astype(np.float32)
